# revision 5
# baseline (speedup 1.0000x reference)
"""Trainium2 Bass kernel for nn_BiSRConv2d_Down.

Reference semantics (forward values):
  out  = avgpool2x2(x)                                  [B, C, H/2, W/2]
  for branch b in {1, 2}:
    xb   = sign(out * mvk_b + mvb_b)                    (tanh STE terms cancel)
    bw   = mean|W_b|_(i,kh,kw) * sign(W_b)              per-output-channel scale
    conv = conv2d(xb, bw, pad=1)
    y_b  = out + (prelu(conv + pb0_b; alpha_b) + pb1_b)
  return concat([y1, y2], channel axis)

Strategy: data-parallel over batch on 8 cores (2 images/core). Per image the
conv is 9 shifted 128x128 matmuls (channels on partitions) accumulated in
PSUM; sign activations are exactly representable in bf16 so the matmul
accumulates exact integer sums. Per-channel params ride as per-partition
scalar operands of ScalarE/VectorE ops. PReLU uses prelu(v) = max(v, alpha*v)
(valid for alpha in [0, 1], which holds per-channel here; checked on host).
"""

import numpy as np

import concourse.bacc as bacc
import concourse.mybir as mybir
import concourse.tile as tile
from concourse.bass_utils import run_bass_kernel_spmd

F32 = mybir.dt.float32
BF16 = mybir.dt.bfloat16
AF = mybir.ActivationFunctionType
ALU = mybir.AluOpType

B, C, H, W = 16, 128, 128, 128
NCORES = 8
IPC = B // NCORES          # images per core
HP, WP = H // 2, W // 2    # pooled height/width: 64, 64
RL = WP + 2                # padded row stride 66
NPADF = (HP + 2) * RL      # padded image size 4356
XBPLEN = NPADF + 2         # +2 tail pad so tap reads stay in-bounds
NROWCH = 7                 # output rows per PSUM chunk (7*66=462 <= 512)
# (q0, nrows, ncols) per PSUM chunk; the tail chunk is the last single row
CHUNKS = [(ci * NROWCH * RL, NROWCH, NROWCH * RL) for ci in range(9)]
CHUNKS.append((63 * RL, 1, RL))
POOLCH = 8                 # pooled rows per input chunk (16 x rows -> 1 MiB DMA)
NCH = HP // POOLCH         # input/pool/sign chunks per image


def build_nc():
    nc = bacc.Bacc(
        "TRN2", target_bir_lowering=False, debug=False, num_devices=NCORES
    )
    x_d = nc.dram_tensor("x", [IPC, C, H, W], F32, kind="ExternalInput")
    # wt: host-relaid weights, wt[b][i, t*128+o] = W_b[o, i, kh, kw], t=kh*3+kw
    wt_d = nc.dram_tensor("wt", [2, C, 9 * C], F32, kind="ExternalInput")
    # wn: natural weights flattened per output channel (for mean|W|)
    wn_d = nc.dram_tensor("wn", [2, C, 9 * C], F32, kind="ExternalInput")
    # pp: per-channel params, col 5*b+{0:mvk,1:mvb,2:pb0,3:alpha,4:pb1}
    pp_d = nc.dram_tensor("pp", [C, 10], F32, kind="ExternalInput")
    out_d = nc.dram_tensor("out", [IPC, 2 * C, HP, WP], F32, kind="ExternalOutput")

    with tile.TileContext(nc) as tc:
        with (
            tc.tile_pool(name="const", bufs=1) as cpool,
            tc.tile_pool(name="wload", bufs=2) as wpool,
            tc.tile_pool(name="xin", bufs=3) as xpool,
            tc.tile_pool(name="t1p", bufs=2) as t1pool,
            tc.tile_pool(name="abuf", bufs=2) as apool,
            tc.tile_pool(name="oasm", bufs=2) as opool,
            tc.tile_pool(name="resp", bufs=1) as respool,
            tc.tile_pool(name="ps", bufs=7, space="PSUM") as pspool,
        ):
            # ---------- weight / param prep ----------
            pp_t = cpool.tile([C, 10], F32, name="pp_t")
            nc.sync.dma_start(pp_t[:], pp_d[:])

            sgn, scale_w, sc_sign = [], [], []
            for b in range(2):
                wld = wpool.tile([C, 9 * C], F32, tag="wload", name=f"wld{b}")
                nc.sync.dma_start(wld[:], wt_d[b])
                s = cpool.tile([C, 9 * C], BF16, name=f"sgnw{b}")
                nc.scalar.sign(s[:], wld[:])
                sgn.append(s)

                wnl = wpool.tile([C, 9 * C], F32, tag="wload", name=f"wnl{b}")
                nc.sync.dma_start(wnl[:], wn_d[b])
                asum = cpool.tile([C, 1], F32, name=f"asum{b}")
                nc.scalar.activation(wnl[:], wnl[:], AF.Abs, accum_out=asum[:])
                sw = cpool.tile([C, 1], F32, name=f"scale_w{b}")
                nc.vector.tensor_scalar_mul(sw[:], asum[:], 1.0 / (9 * C))
                scale_w.append(sw)

                ss = cpool.tile([C, 1], F32, name=f"sc_sign{b}")
                nc.vector.tensor_scalar_mul(
                    ss[:], pp_t[:, 5 * b + 0 : 5 * b + 1], 0.25
                )
                sc_sign.append(ss)

            # padded sign-activation buffers (zero borders persist; interiors
            # are fully rewritten per image)
            xbp = [
                [cpool.tile([C, XBPLEN], BF16, name=f"xbp{i}{b}") for b in range(2)]
                for i in range(IPC)
            ]
            for i in range(IPC):
                for b in range(2):
                    nc.gpsimd.memset(xbp[i][b][:], 0.0)

            pooled = [
                cpool.tile([C, HP, WP], F32, name=f"pooled{i}") for i in range(IPC)
            ]

            for i in range(IPC):
                # ---------- avg-pool (sum of 4; the 0.25 is folded later) ----
                for k in range(NCH):
                    xr = xpool.tile(
                        [C, 2 * POOLCH, W], F32, tag="xr", name=f"xr{i}_{k}"
                    )
                    nc.sync.dma_start(
                        xr[:], x_d[i][:, 2 * POOLCH * k : 2 * POOLCH * (k + 1), :]
                    )
                    xr4 = xr[:].rearrange("p r (w two) -> p r w two", two=2)
                    t1 = t1pool.tile(
                        [C, 2 * POOLCH, WP], F32, tag="t1", name=f"t1_{i}_{k}"
                    )
                    nc.vector.tensor_tensor(
                        t1[:], xr4[:, :, :, 0], xr4[:, :, :, 1], ALU.add
                    )
                    t1r = t1[:].rearrange("p (h two) w -> p h two w", two=2)
                    eng2 = nc.gpsimd if k % 2 == 0 else nc.vector
                    eng2.tensor_tensor(
                        pooled[i][:, POOLCH * k : POOLCH * (k + 1), :],
                        t1r[:, :, 0, :],
                        t1r[:, :, 1, :],
                        ALU.add,
                    )
                    # ---------- binary activation: sign(mvk*pool + mvb) -----
                    for b in range(2):
                        xb3 = xbp[i][b][:, :NPADF].rearrange(
                            "p (r c) -> p r c", c=RL
                        )
                        nc.scalar.activation(
                            xb3[:, 1 + POOLCH * k : 1 + POOLCH * (k + 1), 1 : 1 + WP],
                            pooled[i][:, POOLCH * k : POOLCH * (k + 1), :],
                            AF.Sign,
                            bias=pp_t[:, 5 * b + 1 : 5 * b + 2],
                            scale=sc_sign[b][:],
                        )

                # ---------- conv + epilogue per branch ----------
                for b in range(2):
                    # residual + pb1: 0.25 * (sum of 4) + pb1_b
                    rs = respool.tile(
                        [C, HP, WP], F32, tag="res", name=f"res{i}{b}"
                    )
                    nc.vector.tensor_scalar(
                        rs[:], pooled[i][:], 0.25,
                        pp_t[:, 5 * b + 4 : 5 * b + 5], ALU.mult, ALU.add,
                    )
                    ab = apool.tile([C, HP, RL], F32, tag="abuf", name=f"ab{i}{b}")
                    for half in range(2):
                        cs = CHUNKS[5 * half : 5 * half + 5]
                        pts = [
                            pspool.tile(
                                [C, NROWCH * RL], F32, tag="ps",
                                name=f"ps{i}{b}{half}{ci}",
                            )
                            for ci in range(len(cs))
                        ]
                        for t in range(9):
                            off = (t // 3) * RL + (t % 3)
                            lhs = sgn[b][:, C * t : C * (t + 1)]
                            for ci, (q0, nrows, ncols) in enumerate(cs):
                                nc.tensor.matmul(
                                    pts[ci][:, :ncols],
                                    lhs,
                                    xbp[i][b][:, q0 + off : q0 + off + ncols],
                                    start=(t == 0),
                                    stop=(t == 8),
                                )
                        # evict PSUM with fused affine: a = scale_w*S + pb0
                        for ci, (q0, nrows, ncols) in enumerate(cs):
                            r0 = q0 // RL
                            nc.scalar.activation(
                                ab[:, r0 : r0 + nrows, :],
                                pts[ci][:, :ncols].rearrange(
                                    "p (r c) -> p r c", c=RL
                                ),
                                AF.Identity,
                                bias=pp_t[:, 5 * b + 2 : 5 * b + 3],
                                scale=scale_w[b][:],
                            )
                    # prelu(v) = max(v, alpha*v), in place (DVE-only op)
                    nc.vector.scalar_tensor_tensor(
                        ab[:], ab[:], pp_t[:, 5 * b + 3 : 5 * b + 4], ab[:],
                        ALU.mult, ALU.max,
                    )
                    # out = prelu + (residual + pb1)
                    oa = opool.tile([C, HP, WP], F32, tag="oasm", name=f"oa{i}{b}")
                    nc.gpsimd.tensor_tensor(
                        oa[:], ab[:, :, :WP], rs[:], ALU.add
                    )
                    nc.scalar.dma_start(out_d[i, C * b : C * (b + 1), :, :], oa[:])

    nc.compile()
    return nc


def _prep_weights(Wb):
    Wb = np.asarray(Wb, dtype=np.float32)
    wn = Wb.reshape(C, C * 9)
    wt = np.ascontiguousarray(
        Wb.reshape(C, C, 9).transpose(1, 2, 0).reshape(C, 9 * C)
    )
    return wt, wn


def _prep_inputs(inputs):
    x = np.ascontiguousarray(np.asarray(inputs["x"], dtype=np.float32))
    wt1, wn1 = _prep_weights(inputs["W1"])
    wt2, wn2 = _prep_weights(inputs["W2"])
    wt = np.ascontiguousarray(np.stack([wt1, wt2]))
    wn = np.ascontiguousarray(np.stack([wn1, wn2]))

    def col(v):
        return np.asarray(v, dtype=np.float32).reshape(C)

    pp = np.zeros((C, 10), dtype=np.float32)
    for b, sfx in enumerate(("1", "2")):
        pp[:, 5 * b + 0] = col(inputs["mvk" + sfx])
        pp[:, 5 * b + 1] = col(inputs["mvb" + sfx])
        pp[:, 5 * b + 2] = col(inputs["pb0_" + sfx])
        pp[:, 5 * b + 3] = col(inputs["alpha" + sfx])
        pp[:, 5 * b + 4] = col(inputs["pb1_" + sfx])
        a = pp[:, 5 * b + 3]
        assert np.all((a >= 0.0) & (a <= 1.0)), (
            "prelu max-identity requires alpha in [0,1]"
        )

    in_maps = [
        {"x": np.ascontiguousarray(x[IPC * c : IPC * (c + 1)]),
         "wt": wt, "wn": wn, "pp": pp}
        for c in range(NCORES)
    ]
    return in_maps


_NC_CACHE = {}


def get_nc():
    if "nc" not in _NC_CACHE:
        _NC_CACHE["nc"] = build_nc()
    return _NC_CACHE["nc"]


def kernel(__trace__=False, **inputs):
    nc = get_nc()
    in_maps = _prep_inputs(inputs)
    res = run_bass_kernel_spmd(
        nc, in_maps, list(range(NCORES)), trace=bool(__trace__)
    )
    out = np.concatenate([res.results[c]["out"] for c in range(NCORES)], axis=0)
    out = np.ascontiguousarray(out.astype(np.float32))
    if __trace__:
        return out, res
    return out


# revision 6
# speedup vs baseline: 1.0911x; 1.0911x over previous
"""Trainium2 Bass kernel for nn_BiSRConv2d_Down.

Reference semantics (forward values):
  out  = avgpool2x2(x)                                  [B, C, H/2, W/2]
  for branch b in {1, 2}:
    xb   = sign(out * mvk_b + mvb_b)                    (tanh STE terms cancel)
    bw   = mean|W_b|_(i,kh,kw) * sign(W_b)              per-output-channel scale
    conv = conv2d(xb, bw, pad=1)
    y_b  = out + (prelu(conv + pb0_b; alpha_b) + pb1_b)
  return concat([y1, y2], channel axis)

Strategy: data-parallel over batch on 8 cores (2 images/core). Per image the
conv is 9 shifted 128x128 matmuls (channels on partitions) accumulated in
PSUM chunk-major (9 taps back-to-back per PSUM chunk; LDWEIGHTS overlaps via
the PE weight double-buffer); sign activations are exactly representable in
bf16 so the matmuls accumulate exact integer sums. Per-channel params ride as
per-partition scalar operands. PReLU uses prelu(v) = max(v, alpha*v) (valid
for alpha in [0, 1], which holds per-channel here; checked on host).
"""

import numpy as np

import concourse.bacc as bacc
import concourse.mybir as mybir
import concourse.tile as tile
from concourse.bass_utils import run_bass_kernel_spmd

F32 = mybir.dt.float32
BF16 = mybir.dt.bfloat16
AF = mybir.ActivationFunctionType
ALU = mybir.AluOpType

B, C, H, W = 16, 128, 128, 128
NCORES = 8
IPC = B // NCORES          # images per core
HP, WP = H // 2, W // 2    # pooled height/width: 64, 64
RL = WP + 2                # padded row stride 66
NPADF = (HP + 2) * RL      # padded image size 4356
XBPLEN = NPADF + 2         # +2 tail pad so tap reads stay in-bounds
NROWCH = 7                 # output rows per PSUM chunk (7*66=462 <= 512)
# (q0, nrows, ncols) per PSUM chunk; the tail chunk is the last single row
CHUNKS = [(ci * NROWCH * RL, NROWCH, NROWCH * RL) for ci in range(9)]
CHUNKS.append((63 * RL, 1, RL))
# epilogue halves: output row ranges
HALVES = [(0, 35), (35, 29)]
POOLCH = 8                 # pooled rows per input chunk (16 x rows -> 1 MiB DMA)
NCH = HP // POOLCH         # input/pool/sign chunks per image


def build_nc():
    nc = bacc.Bacc(
        "TRN2", target_bir_lowering=False, debug=False, num_devices=NCORES
    )
    x_d = nc.dram_tensor("x", [IPC, C, H, W], F32, kind="ExternalInput")
    # wt: host-relaid weights, wt[b][i, t*128+o] = W_b[o, i, kh, kw], t=kh*3+kw
    wt_d = nc.dram_tensor("wt", [2, C, 9 * C], F32, kind="ExternalInput")
    # wn: natural weights flattened per output channel (for mean|W|)
    wn_d = nc.dram_tensor("wn", [2, C, 9 * C], F32, kind="ExternalInput")
    # pp: per-channel params, col 5*b+{0:mvk,1:mvb,2:pb0,3:alpha,4:pb1}
    pp_d = nc.dram_tensor("pp", [C, 10], F32, kind="ExternalInput")
    out_d = nc.dram_tensor("out", [IPC, 2 * C, HP, WP], F32, kind="ExternalOutput")

    with tile.TileContext(nc) as tc:
        with (
            tc.tile_pool(name="const", bufs=1) as cpool,
            tc.tile_pool(name="wload", bufs=1) as wpool,
            tc.tile_pool(name="xin", bufs=3) as xpool,
            tc.tile_pool(name="t1p", bufs=2) as t1pool,
            tc.tile_pool(name="oasm", bufs=2) as opool,
            tc.tile_pool(name="resp", bufs=2) as respool,
            tc.tile_pool(name="ps", bufs=6, space="PSUM") as pspool,
        ):
            # ---------- weight / param prep ----------
            pp_t = cpool.tile([C, 10], F32, name="pp_t")
            nc.sync.dma_start(pp_t[:], pp_d[:])

            sgn, scale_w, sc_sign = [], [], []
            for b in range(2):
                wld = wpool.tile([C, 9 * C], F32, tag="wload", name=f"wld{b}")
                nc.sync.dma_start(wld[:], wt_d[b])
                s = cpool.tile([C, 9 * C], BF16, name=f"sgnw{b}")
                nc.scalar.sign(s[:], wld[:])
                sgn.append(s)

                wnl = wpool.tile([C, 9 * C], F32, tag="wload", name=f"wnl{b}")
                nc.sync.dma_start(wnl[:], wn_d[b])
                asum = cpool.tile([C, 1], F32, name=f"asum{b}")
                nc.scalar.activation(wnl[:], wnl[:], AF.Abs, accum_out=asum[:])
                sw = cpool.tile([C, 1], F32, name=f"scale_w{b}")
                nc.vector.tensor_scalar_mul(sw[:], asum[:], 1.0 / (9 * C))
                scale_w.append(sw)

                ss = cpool.tile([C, 1], F32, name=f"sc_sign{b}")
                nc.vector.tensor_scalar_mul(
                    ss[:], pp_t[:, 5 * b + 0 : 5 * b + 1], 0.25
                )
                sc_sign.append(ss)

            # padded sign-activation buffers: only the BORDERS need zeroing
            # (row 0, row 65, cols 0/65 of each row, 2-elem tail); interiors
            # are fully rewritten per image.
            xbp = [
                [cpool.tile([C, XBPLEN], BF16, name=f"xbp{i}{b}") for b in range(2)]
                for i in range(IPC)
            ]
            for i in range(IPC):
                for b in range(2):
                    t = xbp[i][b]
                    nc.vector.memset(t[:, 0:67], 0.0)
                    edge = t[:, 65 : 65 + 65 * RL].rearrange(
                        "p (r c) -> p r c", c=RL
                    )
                    nc.vector.memset(edge[:, :, 0:2], 0.0)
                    nc.vector.memset(t[:, 65 * RL : XBPLEN], 0.0)

            pooled = [
                cpool.tile([C, HP, WP], F32, name=f"pooled{i}") for i in range(IPC)
            ]

            for i in range(IPC):
                # ---------- avg-pool (sum of 4; the 0.25 is folded later) ----
                for k in range(NCH):
                    xr = xpool.tile(
                        [C, 2 * POOLCH, W], F32, tag="xr", name=f"xr{i}_{k}"
                    )
                    nc.sync.dma_start(
                        xr[:], x_d[i][:, 2 * POOLCH * k : 2 * POOLCH * (k + 1), :]
                    )
                    # row pairs first: contiguous innermost reads on DVE
                    xrr = xr[:].rearrange("p (h two) w -> p h two w", two=2)
                    t1 = t1pool.tile(
                        [C, POOLCH, W], F32, tag="t1", name=f"t1_{i}_{k}"
                    )
                    nc.vector.tensor_tensor(
                        t1[:], xrr[:, :, 0, :], xrr[:, :, 1, :], ALU.add
                    )
                    # then adjacent-column pairs (strided) on GpSimd
                    t1w = t1[:].rearrange("p h (w two) -> p h w two", two=2)
                    nc.gpsimd.tensor_tensor(
                        pooled[i][:, POOLCH * k : POOLCH * (k + 1), :],
                        t1w[:, :, :, 0],
                        t1w[:, :, :, 1],
                        ALU.add,
                    )
                    # ---------- binary activation: sign(mvk*pool + mvb) -----
                    for b in range(2):
                        xb3 = xbp[i][b][:, :NPADF].rearrange(
                            "p (r c) -> p r c", c=RL
                        )
                        nc.scalar.activation(
                            xb3[:, 1 + POOLCH * k : 1 + POOLCH * (k + 1), 1 : 1 + WP],
                            pooled[i][:, POOLCH * k : POOLCH * (k + 1), :],
                            AF.Sign,
                            bias=pp_t[:, 5 * b + 1 : 5 * b + 2],
                            scale=sc_sign[b][:],
                        )

                # ---------- conv + epilogue per branch ----------
                for b in range(2):
                    # residual + pb1: 0.25 * (sum of 4) + pb1_b
                    rs = respool.tile(
                        [C, HP, WP], F32, tag="res", name=f"res{i}{b}"
                    )
                    nc.gpsimd.tensor_scalar(
                        rs[:], pooled[i][:], 0.25,
                        pp_t[:, 5 * b + 4 : 5 * b + 5], ALU.mult, ALU.add,
                    )

                    oa = opool.tile([C, HP, WP], F32, tag="oasm", name=f"oa{i}{b}")
                    # chunk-major: 9 taps accumulate into one PSUM bank, then
                    # ScalarE evicts valid columns with the fused affine
                    # a = scale_w * S + pb0 straight into the compact buffer.
                    for ci, (q0, nrows, ncols) in enumerate(CHUNKS):
                        pt = pspool.tile(
                            [C, NROWCH * RL], F32, tag="ps", name=f"ps{i}{b}{ci}"
                        )
                        for t in range(9):
                            off = (t // 3) * RL + (t % 3)
                            nc.tensor.matmul(
                                pt[:, :ncols],
                                sgn[b][:, C * t : C * (t + 1)],
                                xbp[i][b][:, q0 + off : q0 + off + ncols],
                                start=(t == 0),
                                stop=(t == 8),
                            )
                        r0 = q0 // RL
                        nc.scalar.activation(
                            oa[:, r0 : r0 + nrows, :],
                            pt[:, :ncols].rearrange("p (r c) -> p r c", c=RL)[
                                :, :, :WP
                            ],
                            AF.Identity,
                            bias=pp_t[:, 5 * b + 2 : 5 * b + 3],
                            scale=scale_w[b][:],
                        )
                    # per-half epilogue: prelu in place, then + residual,
                    # then store
                    for hi, (r0, nr) in enumerate(HALVES):
                        part = oa[:, r0 : r0 + nr, :]
                        nc.vector.scalar_tensor_tensor(
                            part, part, pp_t[:, 5 * b + 3 : 5 * b + 4], part,
                            ALU.mult, ALU.max,
                        )
                        eng = nc.vector if (2 * i + b + hi) % 2 else nc.gpsimd
                        eng.tensor_tensor(
                            part, part, rs[:, r0 : r0 + nr, :], ALU.add
                        )
                        nc.scalar.dma_start(
                            out_d[i, C * b : C * (b + 1), r0 : r0 + nr, :], part
                        )

    nc.compile()
    return nc


def _prep_weights(Wb):
    Wb = np.asarray(Wb, dtype=np.float32)
    wn = Wb.reshape(C, C * 9)
    wt = np.ascontiguousarray(
        Wb.reshape(C, C, 9).transpose(1, 2, 0).reshape(C, 9 * C)
    )
    return wt, wn


def _prep_inputs(inputs):
    x = np.ascontiguousarray(np.asarray(inputs["x"], dtype=np.float32))
    wt1, wn1 = _prep_weights(inputs["W1"])
    wt2, wn2 = _prep_weights(inputs["W2"])
    wt = np.ascontiguousarray(np.stack([wt1, wt2]))
    wn = np.ascontiguousarray(np.stack([wn1, wn2]))

    def col(v):
        return np.asarray(v, dtype=np.float32).reshape(C)

    pp = np.zeros((C, 10), dtype=np.float32)
    for b, sfx in enumerate(("1", "2")):
        pp[:, 5 * b + 0] = col(inputs["mvk" + sfx])
        pp[:, 5 * b + 1] = col(inputs["mvb" + sfx])
        pp[:, 5 * b + 2] = col(inputs["pb0_" + sfx])
        pp[:, 5 * b + 3] = col(inputs["alpha" + sfx])
        pp[:, 5 * b + 4] = col(inputs["pb1_" + sfx])
        a = pp[:, 5 * b + 3]
        assert np.all((a >= 0.0) & (a <= 1.0)), (
            "prelu max-identity requires alpha in [0,1]"
        )

    in_maps = [
        {"x": np.ascontiguousarray(x[IPC * c : IPC * (c + 1)]),
         "wt": wt, "wn": wn, "pp": pp}
        for c in range(NCORES)
    ]
    return in_maps


_NC_CACHE = {}


def get_nc():
    if "nc" not in _NC_CACHE:
        _NC_CACHE["nc"] = build_nc()
    return _NC_CACHE["nc"]


def kernel(__trace__=False, **inputs):
    nc = get_nc()
    in_maps = _prep_inputs(inputs)
    res = run_bass_kernel_spmd(
        nc, in_maps, list(range(NCORES)), trace=bool(__trace__)
    )
    out = np.concatenate([res.results[c]["out"] for c in range(NCORES)], axis=0)
    out = np.ascontiguousarray(out.astype(np.float32))
    if __trace__:
        return out, res
    return out
